# revision 33
# baseline (speedup 1.0000x reference)
"""Trainium2 Bass kernel for nn_AlignBinary (token-equality similarity).

Reference semantics: with emb_weight fixed to the identity matrix, the
one-hot bmm + mask reduces exactly to

    out[b, q, c] = 1.0 if (qry[b,q] == cnd[b,c] and qry[b,q] > 0) else 0.0

Strategy (pure data parallel, batch B=128 split over 8 cores, 16 each):
  - host stages per-core inputs as f32 (exact for ids < 2^24): qryT
    [128, 16] (token q on partitions) and one row [1, 2176] holding the
    16 cnd rows (2048) plus a ones(128) vector.
  - device remaps qry zeros to -1 (qry' = qry - (qry==0)); a single
    is_equal(qry'[q], cnd[c]) then realizes sim * mask (a -1 never
    matches a cnd value in [0, 1023], and equal nonzero pairs imply both
    masks set).
  - PE broadcasts the cnd rows to all 128 partitions with 4 K=1
    ones-outer-product matmuls (N=512 each -> one PSUM bank per group of
    4 batches).
  - DVE compares each PSUM bank against the per-partition qry' column
    (free-dim 0-stride broadcast) -> 4 wide is_equal ops.
  - 4 output DMAs of 4 batches each overlap the compute.

Raw bass (no TileContext, no nc.Block): the Tile/Block entry/exit
all-engine barriers and body branches cost several us on a ~10 us
kernel, and the manual semaphore schedule here is simple: two DMA-in
sems, a PE sem, a DVE sem, a DVE-internal RAW sem, a DMA-out sem.
"""

import numpy as np

B = 128
L = 128
N_CORES = 8
B_LOC = B // N_CORES    # 16 batches per core
NG = 4                  # batch groups (one PSUM bank each)
GSZ = B_LOC // NG       # 4 batches per group
ROWW = B_LOC * L + L    # 2176: cnd rows + ones

_CACHE: dict = {}


def _build_nc():
    import concourse.bass as bass
    import concourse.mybir as mybir

    dt = mybir.dt
    nc = bass.Bass(trn_type="TRN2", name="align_binary")

    qt_d = nc.dram_tensor("qt", [L, B_LOC], dt.float32, kind="ExternalInput")
    # fp16: ids <= 1023 are exact, and fp16 matmuls are single-pass on the
    # PE (f32 matmuls decompose into two bf16 passes, 2x LDW+MM cost).
    row_d = nc.dram_tensor("row", [1, ROWW], dt.float16, kind="ExternalInput")
    out_d = nc.dram_tensor("out", [B_LOC, L, L], dt.float32, kind="ExternalOutput")

    with (
        nc.sbuf_tensor([L, B_LOC], dt.float32) as qts,
        nc.sbuf_tensor([1, ROWW], dt.float16) as rowt,
        nc.sbuf_tensor([L, B_LOC], dt.float32) as qp,
        nc.sbuf_tensor([L, B_LOC * L], dt.float32) as out_sb,
        nc.psum_tensor([L, NG, GSZ * L], dt.float32) as bc,
        nc.semaphore() as s_in,
        nc.semaphore() as s_inq,
        nc.semaphore() as s_pe,
        nc.semaphore() as s_dv,
        nc.semaphore() as s_out,
        nc.semaphore() as s_q,
    ):
        # No nc.Block(): the kernel is branch-free, so every instruction is
        # emitted straight into the main basic block (each engine executes
        # its own subsequence). This drops the per-engine body branches,
        # the empty end block, and the Block exit barrier entirely.
        ones_ap = rowt[0:1, B_LOC * L : B_LOC * L + L]

        def _out_dma(eng, b0, nb):
            src = out_sb[:, b0 * L : (b0 + nb) * L].rearrange(
                "q (b c) -> q b c", b=nb
            )
            dst = out_d[b0 : b0 + nb].rearrange("b q c -> q b c")
            eng.dma_start(dst, src).then_inc(s_out, 16)

        N_OUT_DMA = NG

        # --- input DMAs. row (which gates the PE) rides gpsimd/SWDGE:
        # the Pool engine is otherwise idle and clears the runtime entry
        # sequence earliest. qt rides scalar; sync eats a ~0.7us runtime
        # drain before its first dispatch, so it only handles outputs.
        nc.gpsimd.dma_start(rowt[:], row_d[:]).then_inc(s_in, 16)
        nc.scalar.dma_start(qts[:], qt_d[:]).then_inc(s_inq, 16)

        # --- PE: broadcast cnd rows via K=1 ones outer products ---
        nc.tensor.wait_ge(s_in, 16)
        for g in range(NG):
            # bc[q, g, :] = ones[q] * cnd_rows[g*512:(g+1)*512]
            nc.tensor.matmul(
                bc[:, g, :],
                lhsT=ones_ap,
                rhs=rowt[0:1, g * GSZ * L : (g + 1) * GSZ * L],
                start=True,
                stop=True,
            ).then_inc(s_pe, 1)

        # --- DVE: fused qry' prep + wide is_equal per PSUM bank ---
        nc.vector.wait_ge(s_inq, 16)
        # nqp = (qry == 0) - qry = -qry'. The PE side uses a -1s vector, so
        # bc holds -cnd, and is_equal(-cnd, -qry') is the same predicate.
        # One fused op: no same-engine RAW chain, no extra semaphore.
        nc.vector.scalar_tensor_tensor(
            out=qp[:], in0=qts[:], scalar=0.0, in1=qts[:],
            op0=mybir.AluOpType.is_equal, op1=mybir.AluOpType.subtract,
        ).then_inc(s_q, 1)
        nc.vector.wait_ge(s_q, 1)
        for g in range(NG):
            nc.vector.wait_ge(s_pe, g + 1)
            # out[q, b, c] = (bc[q, b, c] == qry'[q, b])  [b broadcast 128x]
            # AP steps/offsets are in elements; partition dim first.
            in1 = bass.AP(qp, g * GSZ, [[B_LOC, L], [1, GSZ], [0, L]])
            nc.vector.tensor_tensor(
                out=out_sb[:, g * GSZ * L : (g + 1) * GSZ * L].rearrange(
                    "q (b c) -> q b c", b=GSZ
                ),
                in0=bc[:, g, :].rearrange("q (b c) -> q b c", b=GSZ),
                in1=in1,
                op=mybir.AluOpType.is_equal,
            ).then_inc(s_dv, 1)

        # --- output DMAs: one per group, alternating engines. The final
        # group stays whole: an extra dispatch (~0.7us busy) costs more
        # exec time than the larger transfer (waits are idle time).
        nc.scalar.wait_ge(s_dv, 1)
        _out_dma(nc.scalar, 0, GSZ)
        nc.sync.wait_ge(s_dv, 2)
        _out_dma(nc.sync, GSZ, GSZ)
        nc.scalar.wait_ge(s_dv, 3)
        _out_dma(nc.scalar, 2 * GSZ, GSZ)
        nc.sync.wait_ge(s_dv, 4)
        _out_dma(nc.sync, 3 * GSZ, GSZ)
        nc.sync.wait_ge(s_out, 16 * N_OUT_DMA)

    _strip_barriers(nc, mybir)
    nc.finalize()
    return nc


def _strip_barriers(nc, mybir):
    """Remove bass's const-ap memsets and the entry/exit all-engine
    barriers (~2 us of exec window). All cross-engine ordering in this
    kernel flows through explicit semaphores; the runtime zero-inits
    semaphores at NEFF load, and SP only halts after s_out confirms the
    output DMAs landed, so neither barrier is load-bearing here."""
    f = nc.m.functions[0]
    drop = ("Memset", "Drain", "EventSemaphore")
    for bi, blk in enumerate(f.blocks):
        if blk.name != "main" and not blk.name.endswith("_end"):
            continue
        keep = []
        in_preamble = blk.name == "main"
        for i in blk.instructions:
            if i.opcode == "DMACopy":
                in_preamble = False  # reached kernel body; keep my own waits
            if (in_preamble or blk.name.endswith("_end")) and i.opcode in drop:
                continue
            keep.append(i)
        if len(keep) != len(blk.instructions):
            f.blocks[bi] = mybir.BasicBlock(name=blk.name, instructions=keep)


def _get_nc():
    if "nc" not in _CACHE:
        _CACHE["nc"] = _build_nc()
    return _CACHE["nc"]


def _pack(q, c):
    """Stage per-core inputs: qryT f32 [L, B_LOC] and the cnd + (-1)s row."""
    maps = []
    neg1 = np.full((L,), -1.0, dtype=np.float16)
    for i in range(N_CORES):
        qs = q[i * B_LOC : (i + 1) * B_LOC]
        cs = c[i * B_LOC : (i + 1) * B_LOC]
        qt = np.ascontiguousarray(qs.T.astype(np.float32))
        row = np.concatenate([cs.astype(np.float16).reshape(-1), neg1])[None, :]
        maps.append({"qt": qt, "row": np.ascontiguousarray(row)})
    return maps


def _run(q, c, **spmd_kwargs):
    """Shard [B, L] inputs over the 8 cores and run the Bass kernel.

    Returns the BassKernelResults (results per core + optional trace info).
    """
    from concourse.bass_utils import run_bass_kernel_spmd

    nc = _get_nc()
    in_maps = _pack(q, c)
    return run_bass_kernel_spmd(nc, in_maps, core_ids=list(range(N_CORES)), **spmd_kwargs)


def kernel(emb_weight=None, qry_lkup=None, cnd_lkup=None, **_ignored):
    q = np.asarray(qry_lkup, dtype=np.int64)
    c = np.asarray(cnd_lkup, dtype=np.int64)
    assert q.shape == (B, L) and c.shape == (B, L)

    res = _run(q, c)
    out = np.concatenate([r["out"] for r in res.results], axis=0)
    return out


# revision 34
# speedup vs baseline: 1.1818x; 1.1818x over previous
"""Trainium2 Bass kernel for nn_AlignBinary (token-equality similarity).

Reference semantics: with emb_weight fixed to the identity matrix, the
one-hot bmm + mask reduces exactly to

    out[b, q, c] = 1.0 if (qry[b,q] == cnd[b,c] and qry[b,q] > 0) else 0.0

Strategy (pure data parallel, batch B=128 split over 8 cores, 16 each):
  - host stages per-core inputs as f32 (exact for ids < 2^24): qryT
    [128, 16] (token q on partitions) and one row [1, 2176] holding the
    16 cnd rows (2048) plus a ones(128) vector.
  - device remaps qry zeros to -1 (qry' = qry - (qry==0)); a single
    is_equal(qry'[q], cnd[c]) then realizes sim * mask (a -1 never
    matches a cnd value in [0, 1023], and equal nonzero pairs imply both
    masks set).
  - PE broadcasts the cnd rows to all 128 partitions with 4 K=1
    ones-outer-product matmuls (N=512 each -> one PSUM bank per group of
    4 batches).
  - DVE compares each PSUM bank against the per-partition qry' column
    (free-dim 0-stride broadcast) -> 4 wide is_equal ops.
  - 4 output DMAs of 4 batches each overlap the compute.

Raw bass (no TileContext, no nc.Block): the Tile/Block entry/exit
all-engine barriers and body branches cost several us on a ~10 us
kernel, and the manual semaphore schedule here is simple: two DMA-in
sems, a PE sem, a DVE sem, a DVE-internal RAW sem, a DMA-out sem.
"""

import numpy as np

B = 128
L = 128
N_CORES = 8
B_LOC = B // N_CORES    # 16 batches per core
NG = 4                  # batch groups (one PSUM bank each)
GSZ = B_LOC // NG       # 4 batches per group
ROWW = B_LOC * L + L    # 2176: cnd rows + ones

_CACHE: dict = {}


def _build_nc():
    import concourse.bass as bass
    import concourse.mybir as mybir

    dt = mybir.dt
    nc = bass.Bass(trn_type="TRN2", name="align_binary")

    qt_d = nc.dram_tensor("qt", [L, B_LOC], dt.float32, kind="ExternalInput")
    # fp16: ids <= 1023 are exact, and fp16 matmuls are single-pass on the
    # PE (f32 matmuls decompose into two bf16 passes, 2x LDW+MM cost).
    row_d = nc.dram_tensor("row", [1, ROWW], dt.float16, kind="ExternalInput")
    out_d = nc.dram_tensor("out", [B_LOC, L, L], dt.float32, kind="ExternalOutput")

    with (
        nc.sbuf_tensor([L, B_LOC], dt.float32) as qts,
        nc.sbuf_tensor([1, ROWW], dt.float16) as rowt,
        nc.sbuf_tensor([L, B_LOC], dt.float32) as qp,
        nc.sbuf_tensor([L, B_LOC * L], dt.float32) as out_sb,
        nc.psum_tensor([L, NG, GSZ * L], dt.float32) as bc,
        nc.semaphore() as s_in,
        nc.semaphore() as s_inq,
        nc.semaphore() as s_pe,
        nc.semaphore() as s_dv,
        nc.semaphore() as s_out,
        nc.semaphore() as s_q,
    ):
        # No nc.Block(): the kernel is branch-free, so every instruction is
        # emitted straight into the main basic block (each engine executes
        # its own subsequence). This drops the per-engine body branches,
        # the empty end block, and the Block exit barrier entirely.
        ones_ap = rowt[0:1, B_LOC * L : B_LOC * L + L]

        def _out_dma(eng, b0, nb):
            src = out_sb[:, b0 * L : (b0 + nb) * L].rearrange(
                "q (b c) -> q b c", b=nb
            )
            dst = out_d[b0 : b0 + nb].rearrange("b q c -> q b c")
            eng.dma_start(dst, src).then_inc(s_out, 16)

        N_OUT_DMA = NG

        # --- input DMAs, dispatched in parallel from both HWDGE engines.
        # row (which gates the PE) rides scalar: sync eats a ~0.7us
        # runtime drain before its first dispatch.
        nc.scalar.dma_start(rowt[:], row_d[:]).then_inc(s_in, 16)
        nc.sync.dma_start(qts[:], qt_d[:]).then_inc(s_inq, 16)

        # --- PE: broadcast cnd rows via K=1 ones outer products ---
        nc.tensor.wait_ge(s_in, 16)
        for g in range(NG):
            # bc[q, g, :] = ones[q] * cnd_rows[g*512:(g+1)*512]
            nc.tensor.matmul(
                bc[:, g, :],
                lhsT=ones_ap,
                rhs=rowt[0:1, g * GSZ * L : (g + 1) * GSZ * L],
                start=True,
                stop=True,
            ).then_inc(s_pe, 1)

        # --- DVE: fused qry' prep + wide is_equal per PSUM bank ---
        nc.vector.wait_ge(s_inq, 16)
        # nqp = (qry == 0) - qry = -qry'. The PE side uses a -1s vector, so
        # bc holds -cnd, and is_equal(-cnd, -qry') is the same predicate.
        # One fused op: no same-engine RAW chain, no extra semaphore.
        nc.vector.scalar_tensor_tensor(
            out=qp[:], in0=qts[:], scalar=0.0, in1=qts[:],
            op0=mybir.AluOpType.is_equal, op1=mybir.AluOpType.subtract,
        ).then_inc(s_q, 1)
        nc.vector.wait_ge(s_q, 1)
        for g in range(NG):
            nc.vector.wait_ge(s_pe, g + 1)
            # out[q, b, c] = (bc[q, b, c] == qry'[q, b])  [b broadcast 128x]
            # AP steps/offsets are in elements; partition dim first.
            in1 = bass.AP(qp, g * GSZ, [[B_LOC, L], [1, GSZ], [0, L]])
            nc.vector.tensor_tensor(
                out=out_sb[:, g * GSZ * L : (g + 1) * GSZ * L].rearrange(
                    "q (b c) -> q b c", b=GSZ
                ),
                in0=bc[:, g, :].rearrange("q (b c) -> q b c", b=GSZ),
                in1=in1,
                op=mybir.AluOpType.is_equal,
            ).then_inc(s_dv, 1)

        # --- output DMAs: one per group, alternating engines. The final
        # group stays whole: an extra dispatch (~0.7us busy) costs more
        # exec time than the larger transfer (waits are idle time).
        nc.scalar.wait_ge(s_dv, 1)
        _out_dma(nc.scalar, 0, GSZ)
        nc.sync.wait_ge(s_dv, 2)
        _out_dma(nc.sync, GSZ, GSZ)
        nc.scalar.wait_ge(s_dv, 3)
        _out_dma(nc.scalar, 2 * GSZ, GSZ)
        nc.sync.wait_ge(s_dv, 4)
        _out_dma(nc.sync, 3 * GSZ, GSZ)
        nc.sync.wait_ge(s_out, 16 * N_OUT_DMA)

    _strip_barriers(nc, mybir)
    nc.finalize()
    return nc


def _strip_barriers(nc, mybir):
    """Remove bass's const-ap memsets and the entry/exit all-engine
    barriers (~2 us of exec window). All cross-engine ordering in this
    kernel flows through explicit semaphores; the runtime zero-inits
    semaphores at NEFF load, and SP only halts after s_out confirms the
    output DMAs landed, so neither barrier is load-bearing here."""
    f = nc.m.functions[0]
    drop = ("Memset", "Drain", "EventSemaphore")
    for bi, blk in enumerate(f.blocks):
        if blk.name != "main" and not blk.name.endswith("_end"):
            continue
        keep = []
        in_preamble = blk.name == "main"
        for i in blk.instructions:
            if i.opcode == "DMACopy":
                in_preamble = False  # reached kernel body; keep my own waits
            if (in_preamble or blk.name.endswith("_end")) and i.opcode in drop:
                continue
            keep.append(i)
        if len(keep) != len(blk.instructions):
            f.blocks[bi] = mybir.BasicBlock(name=blk.name, instructions=keep)


def _get_nc():
    if "nc" not in _CACHE:
        _CACHE["nc"] = _build_nc()
    return _CACHE["nc"]


def _pack(q, c):
    """Stage per-core inputs: qryT f32 [L, B_LOC] and the cnd + (-1)s row."""
    maps = []
    neg1 = np.full((L,), -1.0, dtype=np.float16)
    for i in range(N_CORES):
        qs = q[i * B_LOC : (i + 1) * B_LOC]
        cs = c[i * B_LOC : (i + 1) * B_LOC]
        qt = np.ascontiguousarray(qs.T.astype(np.float32))
        row = np.concatenate([cs.astype(np.float16).reshape(-1), neg1])[None, :]
        maps.append({"qt": qt, "row": np.ascontiguousarray(row)})
    return maps


def _run(q, c, **spmd_kwargs):
    """Shard [B, L] inputs over the 8 cores and run the Bass kernel.

    Returns the BassKernelResults (results per core + optional trace info).
    """
    from concourse.bass_utils import run_bass_kernel_spmd

    nc = _get_nc()
    in_maps = _pack(q, c)
    return run_bass_kernel_spmd(nc, in_maps, core_ids=list(range(N_CORES)), **spmd_kwargs)


def kernel(emb_weight=None, qry_lkup=None, cnd_lkup=None, **_ignored):
    q = np.asarray(qry_lkup, dtype=np.int64)
    c = np.asarray(cnd_lkup, dtype=np.int64)
    assert q.shape == (B, L) and c.shape == (B, L)

    res = _run(q, c)
    out = np.concatenate([r["out"] for r in res.results], axis=0)
    return out


# revision 35
# speedup vs baseline: 1.1824x; 1.0006x over previous
"""Trainium2 Bass kernel for nn_AlignBinary (token-equality similarity).

Reference semantics: with emb_weight fixed to the identity matrix, the
one-hot bmm + mask reduces exactly to

    out[b, q, c] = 1.0 if (qry[b,q] == cnd[b,c] and qry[b,q] > 0) else 0.0

Strategy (pure data parallel, batch B=128 split over 8 cores, 16 each):
  - host stages per-core inputs: qryT f32 [128, 16] (token q on
    partitions; ids < 2^24 are exact in f32) and one fp16 row [1, 2176]
    holding the 16 cnd rows (2048) plus a (-1)s(128) vector (ids < 2048
    are exact in fp16, and fp16 matmuls are single-pass on the PE).
  - device computes nqp = (qry==0) - qry = -qry' in ONE fused DVE op;
    is_equal(-cnd, -qry') realizes sim * mask in a single compare (a
    qry'=-1, i.e. nqp=+1, never matches -cnd in [-1023, 0], and equal
    nonzero pairs imply both masks set).
  - PE broadcasts -cnd to all 128 partitions with 4 K=1 (-1)s
    outer-product matmuls (N=512 each -> one PSUM bank per 4 batches).
  - DVE compares each PSUM bank against the per-partition nqp column
    (free-dim 0-stride broadcast) -> 4 wide is_equal ops.
  - 4 output DMAs of 4 batches each, dispatch alternating between the
    two HWDGE engines (SP and ACT), overlap the compute.

Raw bass (no TileContext, no nc.Block): the Tile/Block entry/exit
all-engine barriers and body branches cost several us on a ~10 us
kernel, and the manual semaphore schedule here is simple: two DMA-in
sems, a PE sem, a DVE sem, a DVE-internal RAW sem, a DMA-out sem.
"""

import numpy as np

B = 128
L = 128
N_CORES = 8
B_LOC = B // N_CORES    # 16 batches per core
NG = 4                  # batch groups (one PSUM bank each)
GSZ = B_LOC // NG       # 4 batches per group
ROWW = B_LOC * L + L    # 2176: cnd rows + ones

_CACHE: dict = {}


def _build_nc():
    import concourse.bass as bass
    import concourse.mybir as mybir

    dt = mybir.dt
    nc = bass.Bass(trn_type="TRN2", name="align_binary")

    qt_d = nc.dram_tensor("qt", [L, B_LOC], dt.float32, kind="ExternalInput")
    # fp16: ids <= 1023 are exact, and fp16 matmuls are single-pass on the
    # PE (f32 matmuls decompose into two bf16 passes, 2x LDW+MM cost).
    row_d = nc.dram_tensor("row", [1, ROWW], dt.float16, kind="ExternalInput")
    out_d = nc.dram_tensor("out", [B_LOC, L, L], dt.float32, kind="ExternalOutput")

    with (
        nc.sbuf_tensor([L, B_LOC], dt.float32) as qts,
        nc.sbuf_tensor([1, ROWW], dt.float16) as rowt,
        nc.sbuf_tensor([L, B_LOC], dt.float32) as qp,
        nc.sbuf_tensor([L, B_LOC * L], dt.float32) as out_sb,
        nc.psum_tensor([L, NG, GSZ * L], dt.float32) as bc,
        nc.semaphore() as s_in,
        nc.semaphore() as s_inq,
        nc.semaphore() as s_pe,
        nc.semaphore() as s_dv,
        nc.semaphore() as s_out,
        nc.semaphore() as s_q,
    ):
        # No nc.Block(): the kernel is branch-free, so every instruction is
        # emitted straight into the main basic block (each engine executes
        # its own subsequence). This drops the per-engine body branches,
        # the empty end block, and the Block exit barrier entirely.
        ones_ap = rowt[0:1, B_LOC * L : B_LOC * L + L]

        def _out_dma(eng, b0, nb):
            src = out_sb[:, b0 * L : (b0 + nb) * L].rearrange(
                "q (b c) -> q b c", b=nb
            )
            dst = out_d[b0 : b0 + nb].rearrange("b q c -> q b c")
            eng.dma_start(dst, src).then_inc(s_out, 16)

        N_OUT_DMA = NG

        # --- input DMAs, dispatched in parallel from both HWDGE engines.
        # row (which gates the PE) rides scalar: sync eats a ~0.7us
        # runtime drain before its first dispatch.
        nc.scalar.dma_start(rowt[:], row_d[:]).then_inc(s_in, 16)
        nc.sync.dma_start(qts[:], qt_d[:]).then_inc(s_inq, 16)

        # --- PE: broadcast cnd rows via K=1 ones outer products ---
        nc.tensor.wait_ge(s_in, 16)
        for g in range(NG):
            # bc[q, g, :] = ones[q] * cnd_rows[g*512:(g+1)*512]
            nc.tensor.matmul(
                bc[:, g, :],
                lhsT=ones_ap,
                rhs=rowt[0:1, g * GSZ * L : (g + 1) * GSZ * L],
                start=True,
                stop=True,
            ).then_inc(s_pe, 1)

        # --- DVE: fused qry' prep + wide is_equal per PSUM bank ---
        nc.vector.wait_ge(s_inq, 16)
        # nqp = (qry == 0) - qry = -qry'. The PE side uses a -1s vector, so
        # bc holds -cnd, and is_equal(-cnd, -qry') is the same predicate.
        # One fused op: no same-engine RAW chain, no extra semaphore.
        nc.vector.scalar_tensor_tensor(
            out=qp[:], in0=qts[:], scalar=0.0, in1=qts[:],
            op0=mybir.AluOpType.is_equal, op1=mybir.AluOpType.subtract,
        ).then_inc(s_q, 1)
        nc.vector.wait_ge(s_q, 1)
        for g in range(NG):
            nc.vector.wait_ge(s_pe, g + 1)
            # out[q, b, c] = (bc[q, b, c] == qry'[q, b])  [b broadcast 128x]
            # AP steps/offsets are in elements; partition dim first.
            in1 = bass.AP(qp, g * GSZ, [[B_LOC, L], [1, GSZ], [0, L]])
            nc.vector.tensor_tensor(
                out=out_sb[:, g * GSZ * L : (g + 1) * GSZ * L].rearrange(
                    "q (b c) -> q b c", b=GSZ
                ),
                in0=bc[:, g, :].rearrange("q (b c) -> q b c", b=GSZ),
                in1=in1,
                op=mybir.AluOpType.is_equal,
            ).then_inc(s_dv, 1)

        # --- output DMAs: one per group, alternating engines. The final
        # group stays whole: an extra dispatch (~0.7us busy) costs more
        # exec time than the larger transfer (waits are idle time).
        nc.scalar.wait_ge(s_dv, 1)
        _out_dma(nc.scalar, 0, GSZ)
        nc.sync.wait_ge(s_dv, 2)
        _out_dma(nc.sync, GSZ, GSZ)
        nc.scalar.wait_ge(s_dv, 3)
        _out_dma(nc.scalar, 2 * GSZ, GSZ)
        nc.sync.wait_ge(s_dv, 4)
        _out_dma(nc.sync, 3 * GSZ, GSZ)
        nc.sync.wait_ge(s_out, 16 * N_OUT_DMA)

    _strip_barriers(nc, mybir)
    nc.finalize()
    return nc


def _strip_barriers(nc, mybir):
    """Remove bass's const-ap memsets and the entry/exit all-engine
    barriers (~2 us of exec window). All cross-engine ordering in this
    kernel flows through explicit semaphores; the runtime zero-inits
    semaphores at NEFF load, and SP only halts after s_out confirms the
    output DMAs landed, so neither barrier is load-bearing here."""
    f = nc.m.functions[0]
    drop = ("Memset", "Drain", "EventSemaphore")
    for bi, blk in enumerate(f.blocks):
        if blk.name != "main" and not blk.name.endswith("_end"):
            continue
        keep = []
        in_preamble = blk.name == "main"
        for i in blk.instructions:
            if i.opcode == "DMACopy":
                in_preamble = False  # reached kernel body; keep my own waits
            if (in_preamble or blk.name.endswith("_end")) and i.opcode in drop:
                continue
            keep.append(i)
        if len(keep) != len(blk.instructions):
            f.blocks[bi] = mybir.BasicBlock(name=blk.name, instructions=keep)


def _get_nc():
    if "nc" not in _CACHE:
        _CACHE["nc"] = _build_nc()
    return _CACHE["nc"]


def _pack(q, c):
    """Stage per-core inputs: qryT f32 [L, B_LOC] and the cnd + (-1)s row."""
    maps = []
    neg1 = np.full((L,), -1.0, dtype=np.float16)
    for i in range(N_CORES):
        qs = q[i * B_LOC : (i + 1) * B_LOC]
        cs = c[i * B_LOC : (i + 1) * B_LOC]
        qt = np.ascontiguousarray(qs.T.astype(np.float32))
        row = np.concatenate([cs.astype(np.float16).reshape(-1), neg1])[None, :]
        maps.append({"qt": qt, "row": np.ascontiguousarray(row)})
    return maps


def _run(q, c, **spmd_kwargs):
    """Shard [B, L] inputs over the 8 cores and run the Bass kernel.

    Returns the BassKernelResults (results per core + optional trace info).
    """
    from concourse.bass_utils import run_bass_kernel_spmd

    nc = _get_nc()
    in_maps = _pack(q, c)
    return run_bass_kernel_spmd(nc, in_maps, core_ids=list(range(N_CORES)), **spmd_kwargs)


def kernel(emb_weight=None, qry_lkup=None, cnd_lkup=None, **_ignored):
    q = np.asarray(qry_lkup, dtype=np.int64)
    c = np.asarray(cnd_lkup, dtype=np.int64)
    assert q.shape == (B, L) and c.shape == (B, L)

    res = _run(q, c)
    out = np.concatenate([r["out"] for r in res.results], axis=0)
    return out


# revision 36
# speedup vs baseline: 1.3174x; 1.1141x over previous
"""Trainium2 Bass kernel for nn_AlignBinary (token-equality similarity).

Reference semantics: with emb_weight fixed to the identity matrix, the
one-hot bmm + mask reduces exactly to

    out[b, q, c] = 1.0 if (qry[b,q] == cnd[b,c] and qry[b,q] > 0) else 0.0

Strategy (pure data parallel, batch B=128 split over 8 cores, 16 each):
  - host stages per-core inputs: qryT f32 [128, 16] (token q on
    partitions; ids < 2^24 are exact in f32) and one fp16 row [1, 2176]
    holding the 16 cnd rows (2048) plus a (-1)s(128) vector (ids < 2048
    are exact in fp16, and fp16 matmuls are single-pass on the PE).
  - device computes nqp = (qry==0) - qry = -qry' in ONE fused DVE op;
    is_equal(-cnd, -qry') realizes sim * mask in a single compare (a
    qry'=-1, i.e. nqp=+1, never matches -cnd in [-1023, 0], and equal
    nonzero pairs imply both masks set).
  - PE broadcasts -cnd to all 128 partitions with 4 K=1 (-1)s
    outer-product matmuls (N=512 each -> one PSUM bank per 4 batches).
  - DVE compares each PSUM bank against the per-partition nqp column
    (free-dim 0-stride broadcast) -> 4 wide is_equal ops.
  - 4 output DMAs of 4 batches each, dispatch alternating between the
    two HWDGE engines (SP and ACT), overlap the compute.

Raw bass (no TileContext, no nc.Block): the Tile/Block entry/exit
all-engine barriers and body branches cost several us on a ~10 us
kernel, and the manual semaphore schedule here is simple: two DMA-in
sems, a PE sem, a DVE sem, a DVE-internal RAW sem, a DMA-out sem.
"""

import numpy as np

B = 128
L = 128
N_CORES = 8
B_LOC = B // N_CORES    # 16 batches per core
NG = 4                  # batch groups (one PSUM bank each)
GSZ = B_LOC // NG       # 4 batches per group
ROWW = B_LOC * L + L    # 2176: cnd rows + ones

_CACHE: dict = {}


def _build_nc():
    import concourse.bass as bass
    import concourse.mybir as mybir

    dt = mybir.dt
    nc = bass.Bass(trn_type="TRN2", name="align_binary")

    qt_d = nc.dram_tensor("qt", [L, B_LOC], dt.float32, kind="ExternalInput")
    # fp16: ids <= 1023 are exact, and fp16 matmuls are single-pass on the
    # PE (f32 matmuls decompose into two bf16 passes, 2x LDW+MM cost).
    row_d = nc.dram_tensor("row", [1, ROWW], dt.float16, kind="ExternalInput")
    out_d = nc.dram_tensor("out", [B_LOC, L, L], dt.float32, kind="ExternalOutput")

    with (
        nc.sbuf_tensor([L, B_LOC], dt.float32) as qts,
        nc.sbuf_tensor([1, ROWW], dt.float16) as rowt,
        nc.sbuf_tensor([L, B_LOC], dt.float32) as qp,
        nc.sbuf_tensor([L, B_LOC * L], dt.float32) as out_sb,
        nc.psum_tensor([L, NG, GSZ * L], dt.float32) as bc,
        nc.semaphore() as s_in,
        nc.semaphore() as s_inq,
        nc.semaphore() as s_pe,
        nc.semaphore() as s_dv,
        nc.semaphore() as s_out,
        nc.semaphore() as s_q,
    ):
        # No nc.Block(): the kernel is branch-free, so every instruction is
        # emitted straight into the main basic block (each engine executes
        # its own subsequence). This drops the per-engine body branches,
        # the empty end block, and the Block exit barrier entirely.
        ones_ap = rowt[0:1, B_LOC * L : B_LOC * L + L]

        def _out_dma(eng, b0, nb):
            src = out_sb[:, b0 * L : (b0 + nb) * L].rearrange(
                "q (b c) -> q b c", b=nb
            )
            dst = out_d[b0 : b0 + nb].rearrange("b q c -> q b c")
            eng.dma_start(dst, src).then_inc(s_out, 16)

        N_OUT_DMA = NG

        # --- input DMAs, dispatched in parallel from both HWDGE engines.
        # row (which gates the PE) rides scalar: sync eats a ~0.7us
        # runtime drain before its first dispatch.
        nc.scalar.dma_start(rowt[:], row_d[:]).then_inc(s_in, 16)
        nc.sync.dma_start(qts[:], qt_d[:]).then_inc(s_inq, 16)

        # --- PE: broadcast cnd rows via K=1 ones outer products ---
        nc.tensor.wait_ge(s_in, 16)
        for g in range(NG):
            # bc[q, g, :] = ones[q] * cnd_rows[g*512:(g+1)*512]
            nc.tensor.matmul(
                bc[:, g, :],
                lhsT=ones_ap,
                rhs=rowt[0:1, g * GSZ * L : (g + 1) * GSZ * L],
                start=True,
                stop=True,
            ).then_inc(s_pe, 1)

        # --- DVE: fused qry' prep + wide is_equal per PSUM bank ---
        nc.vector.wait_ge(s_inq, 16)
        # nqp = (qry == 0) - qry = -qry'. The PE side uses a -1s vector, so
        # bc holds -cnd, and is_equal(-cnd, -qry') is the same predicate.
        # One fused op: no same-engine RAW chain, no extra semaphore.
        nc.vector.scalar_tensor_tensor(
            out=qp[:], in0=qts[:], scalar=0.0, in1=qts[:],
            op0=mybir.AluOpType.is_equal, op1=mybir.AluOpType.subtract,
        ).then_inc(s_q, 1)
        nc.vector.wait_ge(s_q, 1)
        for g in range(NG):
            nc.vector.wait_ge(s_pe, g + 1)
            # out[q, b, c] = (bc[q, b, c] == qry'[q, b])  [b broadcast 128x]
            # AP steps/offsets are in elements; partition dim first.
            in1 = bass.AP(qp, g * GSZ, [[B_LOC, L], [1, GSZ], [0, L]])
            nc.vector.tensor_tensor(
                out=out_sb[:, g * GSZ * L : (g + 1) * GSZ * L].rearrange(
                    "q (b c) -> q b c", b=GSZ
                ),
                in0=bc[:, g, :].rearrange("q (b c) -> q b c", b=GSZ),
                in1=in1,
                op=mybir.AluOpType.is_equal,
            ).then_inc(s_dv, 1)

        # --- output DMAs: one per group, alternating engines. The final
        # group stays whole: an extra dispatch (~0.7us busy) costs more
        # exec time than the larger transfer (waits are idle time).
        nc.scalar.wait_ge(s_dv, 1)
        _out_dma(nc.scalar, 0, GSZ)
        nc.sync.wait_ge(s_dv, 2)
        _out_dma(nc.sync, GSZ, GSZ)
        nc.scalar.wait_ge(s_dv, 3)
        _out_dma(nc.scalar, 2 * GSZ, GSZ)
        nc.sync.wait_ge(s_dv, 4)
        _out_dma(nc.sync, 3 * GSZ, GSZ)
        # completion wait rides scalar (its dispatches end earlier), so
        # sync's stream ends right at its last dispatch
        nc.scalar.wait_ge(s_out, 16 * N_OUT_DMA)

    _strip_barriers(nc, mybir)
    nc.finalize()
    return nc


def _strip_barriers(nc, mybir):
    """Remove bass's const-ap memsets and the entry/exit all-engine
    barriers (~2 us of exec window). All cross-engine ordering in this
    kernel flows through explicit semaphores; the runtime zero-inits
    semaphores at NEFF load, and SP only halts after s_out confirms the
    output DMAs landed, so neither barrier is load-bearing here."""
    f = nc.m.functions[0]
    drop = ("Memset", "Drain", "EventSemaphore")
    for bi, blk in enumerate(f.blocks):
        if blk.name != "main" and not blk.name.endswith("_end"):
            continue
        keep = []
        in_preamble = blk.name == "main"
        for i in blk.instructions:
            if i.opcode == "DMACopy":
                in_preamble = False  # reached kernel body; keep my own waits
            if (in_preamble or blk.name.endswith("_end")) and i.opcode in drop:
                continue
            keep.append(i)
        if len(keep) != len(blk.instructions):
            f.blocks[bi] = mybir.BasicBlock(name=blk.name, instructions=keep)


def _get_nc():
    if "nc" not in _CACHE:
        _CACHE["nc"] = _build_nc()
    return _CACHE["nc"]


def _pack(q, c):
    """Stage per-core inputs: qryT f32 [L, B_LOC] and the cnd + (-1)s row."""
    maps = []
    neg1 = np.full((L,), -1.0, dtype=np.float16)
    for i in range(N_CORES):
        qs = q[i * B_LOC : (i + 1) * B_LOC]
        cs = c[i * B_LOC : (i + 1) * B_LOC]
        qt = np.ascontiguousarray(qs.T.astype(np.float32))
        row = np.concatenate([cs.astype(np.float16).reshape(-1), neg1])[None, :]
        maps.append({"qt": qt, "row": np.ascontiguousarray(row)})
    return maps


def _run(q, c, **spmd_kwargs):
    """Shard [B, L] inputs over the 8 cores and run the Bass kernel.

    Returns the BassKernelResults (results per core + optional trace info).
    """
    from concourse.bass_utils import run_bass_kernel_spmd

    nc = _get_nc()
    in_maps = _pack(q, c)
    return run_bass_kernel_spmd(nc, in_maps, core_ids=list(range(N_CORES)), **spmd_kwargs)


def kernel(emb_weight=None, qry_lkup=None, cnd_lkup=None, **_ignored):
    q = np.asarray(qry_lkup, dtype=np.int64)
    c = np.asarray(cnd_lkup, dtype=np.int64)
    assert q.shape == (B, L) and c.shape == (B, L)

    res = _run(q, c)
    out = np.concatenate([r["out"] for r in res.results], axis=0)
    return out
